# revision 17
# baseline (speedup 1.0000x reference)
"""BERT self-attention kernel for Trainium2, 8-core SPMD.

Problem: hidden_states [S=2048, B=2, H=1024], 16 heads x 64, fp32.
Sharding: core i handles batch b = i//4 and head-group hg = i%4
(4 heads = 256 contiguous columns of Wq/Wk/Wv). Each core:

  hsT   = hs.T                          (PE transposes, f32 -> bf16)
  qT/kT = W.T @ hsT (+bias on DVE)      [d, s] bf16
  v     = hsT.T @ Wv (+bias, K=1 mm)    [t, d] bf16, + ones col
  scT   = kT_h.T @ qT_h                 [t, s] bf16 K=64; both heads of a
                                        pair issued back-to-back at PE
                                        row offsets 0/64 (tile_position)
  expT  = exp(scT / 8)                  (ScalarE, scale fused, bf16 out)
  ctxT_aug = v_aug.T @ expT             [65, s] f32 psum; row 64 = sumexp
  out   = transpose(ctxT_aug)[:, 0:64] * (1 / col 64)

Softmax normalization is deferred past the PV matmul (softmax is
shift-invariant and scores are O(1) here, so no max-subtraction).
The kernel is emitted as a single software pipeline: hs DMA ->
transpose -> K0/Q0 projections -> pair-0 attention (with V and the
pair-1 projections woven into its slack) -> pair-1 attention. The
ScalarE exp stream (~135us) is the critical resource; everything else
hides under it.
"""

import numpy as np

S = 2048
B = 2
H = 1024
NH = 16
HD = 64
P = 128
HG = 256          # head-group width (4 heads) per core
NHEADS_CORE = 4
SBLK = 512        # query block
NB = S // SBLK    # 4
NTCH = S // P     # 16 key chunks
KO = H // P       # 8 contraction chunks for projections
N_CORES = 8

_CACHE = {}


def _build_nc():
    import concourse.mybir as mybir
    import concourse.tile as tile
    from concourse import bacc
    from concourse.masks import make_identity

    f32 = mybir.dt.float32
    bf16 = mybir.dt.bfloat16
    Exp = mybir.ActivationFunctionType.Exp

    nc = bacc.Bacc(None, target_bir_lowering=False)

    hs_d = nc.dram_tensor("hs", [S, H], bf16, kind="ExternalInput")
    wq_d = nc.dram_tensor("wq", [H, HG], bf16, kind="ExternalInput")
    wk_d = nc.dram_tensor("wk", [H, HG], bf16, kind="ExternalInput")
    wv_d = nc.dram_tensor("wv", [H, HG], bf16, kind="ExternalInput")
    bq_d = nc.dram_tensor("bq", [HG], f32, kind="ExternalInput")
    bk_d = nc.dram_tensor("bk", [HG], f32, kind="ExternalInput")
    bv_d = nc.dram_tensor("bv", [HG], f32, kind="ExternalInput")
    ones_d = nc.dram_tensor("ones", [NTCH * NHEADS_CORE * P], bf16, kind="ExternalInput")
    out_d = nc.dram_tensor("out", [S, HG], f32, kind="ExternalOutput")

    with tile.TileContext(nc) as tc:
        with (
            tc.tile_pool(name="const", bufs=1) as cst,
            tc.tile_pool(name="qkv", bufs=1) as qkv,
        ):
            ident = cst.tile([P, P], f32)
            make_identity(nc, ident[:])
            # q/k biases as per-partition columns [128, 2] (chunk m on axis 1)
            bcol_q = cst.tile([P, 2], f32)
            nc.sync.dma_start(bcol_q[:], bq_d.rearrange("(m p) -> p m", p=P))
            bcol_k = cst.tile([P, 2], f32)
            nc.sync.dma_start(bcol_k[:], bk_d.rearrange("(m p) -> p m", p=P))
            # v bias row + bf16 ones row for the K=1 bias matmul
            bv_row = cst.tile([1, HG], bf16)
            nc.gpsimd.dma_start(bv_row[:], bv_d[None, :])
            ones_row = cst.tile([1, P], bf16)
            nc.sync.dma_start(ones_row[:], ones_d[None, 0:P])

            # k0/k1 full [d, s]; q split per s-block; v split by t-group
            qkT = {}
            for nm in ("k0", "k1"):
                qkT[nm] = qkv.tile([P, S], bf16, tag=f"T{nm}", name=f"T{nm}")
            qT_s = {}
            for pair in range(2):
                for si in range(NB):
                    qT_s[(pair, si)] = qkv.tile(
                        [P, SBLK], bf16, tag=f"qT{pair}{si}", name=f"qT{pair}{si}"
                    )
            v_g = []
            for g4 in range(NB):
                vt = qkv.tile([P, 4, NHEADS_CORE, HD + 2], bf16,
                              tag=f"v{g4}", name=f"v{g4}")
                nc.sync.dma_start(
                    vt[:, :, :, HD:HD + 1],
                    ones_d.rearrange("(to h p) -> p to h", p=P, to=NTCH)
                    [:, 4 * g4:4 * g4 + 4, :, None],
                )
                v_g.append(vt)

            # Phase-D pools (allocated up front, released at the end)
            ep = tc.alloc_tile_pool(name="expt", bufs=2)
            op = tc.alloc_tile_pool(name="outs", bufs=3)
            scp = tc.alloc_tile_pool(name="sc_ps", bufs=3, space="PSUM")
            cxp = tc.alloc_tile_pool(name="cx_ps", bufs=2, space="PSUM")

            out_v = out_d.rearrange("(nb c p) hh -> p nb c hh", p=P, c=NB)

            with tc.tile_pool(name="hst", bufs=1) as hstp:
                hsT_q = [hstp.tile([P, KO, SBLK], bf16, tag=f"hsT{si}",
                                   name=f"hsT{si}") for si in range(NB)]
                if True:
                    # hsT loaded directly via XBAR DMA-transpose (bf16)
                    for si in range(NB):
                        for ho in range(KO):
                            nc.sync.dma_start_transpose(
                                hsT_q[si][:, ho, :],
                                hs_d[si * SBLK:(si + 1) * SBLK,
                                     ho * P:(ho + 1) * P],
                            )
                    w_sb = {}
                    for name, wd in (("k", wk_d), ("q", wq_d), ("v", wv_d)):
                        w_sb[name] = cst.tile([P, KO, HG], bf16, tag=f"w{name}",
                                              name=f"w{name}")
                        nc.sync.dma_start(
                            w_sb[name][:], wd.rearrange("(ko p) m -> p ko m", p=P)
                        )

                    Ident = mybir.ActivationFunctionType.Identity

                    def qk_proj(w, bcol, m, si, dst, use_act=False):
                        pst = scp.tile([P, 2, SBLK], f32, tag="sc",
                                       name="qk_ps")[:, 0, :]
                        for ko in range(KO):
                            nc.tensor.matmul(
                                pst,
                                w[:, ko, m * P:(m + 1) * P],
                                hsT_q[si][:, ko, :],
                                start=(ko == 0), stop=(ko == KO - 1),
                            )
                        if use_act:
                            nc.scalar.activation(dst, pst, Ident,
                                                 bias=bcol[:, m:m + 1])
                        else:
                            nc.vector.tensor_scalar_add(dst, pst, bcol[:, m:m + 1])

                    def v_proj(to):
                        pst = scp.tile([P, 2, SBLK], f32, tag="sc",
                                       name="v_ps")[:, 0, 0:HG]
                        for ko in range(KO):
                            nc.tensor.matmul(
                                pst,
                                hsT_q[to // 4][:, ko, (to % 4) * P:(to % 4 + 1) * P],
                                w_sb["v"][:, ko, :],
                                start=(ko == 0), stop=False,
                            )
                        nc.tensor.matmul(
                            pst, ones_row[0:1, :], bv_row[:],
                            start=False, stop=True,
                        )
                        nc.vector.tensor_copy(
                            v_g[to // 4][:, to % 4, :, 0:HD],
                            pst.rearrange("p (h d) -> p h d", d=HD),
                        )

                    # ---- attention pipeline ----------------------------
                    def _attention_pair(pair, unit_hook=None):
                        kTt = qkT[f"k{pair}"]
                        for sb_i in range(NB):
                            qTt = qT_s[(pair, sb_i)]
                            expt = ep.tile([P, NTCH, 2, SBLK], bf16, tag="expt",
                                           name="expt")
                            ctxps = [cxp.tile([HD + 1, SBLK], f32, tag="cx",
                                              name=f"ctx{h2}") for h2 in range(2)]

                            def scores_exp(t):
                                sc = scp.tile([P, 2, SBLK], f32, tag="sc",
                                              name="sc")
                                for h2 in range(2):
                                    po = 64 * h2
                                    nc.tensor.matmul(
                                        sc[:, h2, :],
                                        kTt[po:po + HD, t * P:(t + 1) * P],
                                        qTt[po:po + HD, :],
                                        start=True, stop=True,
                                        tile_position=(po, 0),
                                    )
                                nc.scalar.activation(
                                    expt[:, t, :, :], sc[:], Exp, scale=0.125,
                                )

                            def ctx_batch(ts):
                                for h2 in range(2):
                                    head = pair * 2 + h2
                                    for t in ts:
                                        nc.tensor.matmul(
                                            ctxps[h2][:],
                                            v_g[t // 4][:, t % 4, head, 0:HD + 1],
                                            expt[:, t, h2, :],
                                            start=(t == 0), stop=(t == NTCH - 1),
                                            skip_group_check=True,
                                        )

                            for t in range(NTCH):
                                scores_exp(t)
                                if unit_hook is not None:
                                    unit_hook(sb_i, t)
                                if t in (5, 9, 13):
                                    ctx_batch(range(t - 5, t - 1))
                            ctx_batch(range(12, NTCH))

                            for h2 in range(2):
                                head = pair * 2 + h2
                                ctxT = op.tile([HD + 1, SBLK], f32, tag="ctxT",
                                               name="ctxT")
                                nc.vector.tensor_copy(ctxT[:], ctxps[h2][:])
                                ot = cxp.tile([P, NB, HD + 1], f32, tag="cx",
                                              name="ot")
                                for c in range(NB):
                                    nc.tensor.transpose(
                                        ot[:, c, :],
                                        ctxT[:, c * P:(c + 1) * P],
                                        ident[0:HD + 1, 0:HD + 1],
                                    )
                                rec = op.tile([P, NB, 1], f32, tag="rec",
                                              name="rec")
                                nc.vector.reciprocal(rec[:], ot[:, :, HD:HD + 1])
                                osb = op.tile([P, NB, HD], f32, tag="osb",
                                              name="osb")
                                nc.vector.tensor_tensor(
                                    osb[:], ot[:, :, 0:HD],
                                    rec.to_broadcast([P, NB, HD]),
                                    mybir.AluOpType.mult,
                                )
                                nc.sync.dma_start(
                                    out_v[:, sb_i, :, head * HD:(head + 1) * HD],
                                    osb[:],
                                )

                    # ---- emission ---------------------------------------
                    for si in range(NB):
                        qk_proj(w_sb["k"], bcol_k, 0, si,
                                qkT["k0"][:, si * SBLK:(si + 1) * SBLK],
                                use_act=True)
                        qk_proj(w_sb["q"], bcol_q, 0, si, qT_s[(0, si)][:],
                                use_act=True)

                    def _hook_p0(sb_i, t):
                        if sb_i == 0 and 1 <= t <= 8:
                            # two V chains per unit; group g is complete
                            # before ctx needs it two units later
                            v_proj(2 * (t - 1))
                            v_proj(2 * (t - 1) + 1)
                        elif sb_i == 1 and 1 <= t <= 4:
                            si = t - 1
                            qk_proj(w_sb["k"], bcol_k, 1, si,
                                    qkT["k1"][:, si * SBLK:(si + 1) * SBLK])
                        elif sb_i == 2 and 1 <= t <= 4:
                            qk_proj(w_sb["q"], bcol_q, 1, t - 1, qT_s[(1, t - 1)][:])

                    _attention_pair(0, _hook_p0)
                    _attention_pair(1)

            for _pool in (cxp, scp, op, ep):
                _pool.release()
    nc.compile()
    return nc


def _get_nc():
    if "nc" not in _CACHE:
        _CACHE["nc"] = _build_nc()
    return _CACHE["nc"]


def _kernel_np(hidden_states, attention_mask, Wq, bq, Wk, bk, Wv, bv):
    """Numpy fallback for the general (non-zero attention_mask) case."""
    S_, B_, H_ = hidden_states.shape
    hd = H_ // NH

    def split(x):
        return x.reshape(S_, B_ * NH, hd).transpose(1, 0, 2)

    q = split(hidden_states @ Wq + bq)
    k = split(hidden_states @ Wk + bk)
    v = split(hidden_states @ Wv + bv)
    scores = np.einsum("nsd,ntd->nst", q, k).reshape(B_, NH, S_, S_)
    scores = scores / np.sqrt(np.float32(hd)) + attention_mask
    scores = scores - scores.max(axis=-1, keepdims=True)
    e = np.exp(scores)
    probs = (e / e.sum(axis=-1, keepdims=True)).reshape(B_ * NH, S_, S_)
    ctx = np.einsum("nst,ntd->nsd", probs.astype(np.float32), v)
    return ctx.transpose(1, 0, 2).reshape(S_, B_, H_).astype(np.float32)


def kernel(hidden_states, attention_mask, Wq, bq, Wk, bk, Wv, bv, _trace=False, _tmpdir=None):
    import ml_dtypes
    bf = ml_dtypes.bfloat16
    hidden_states = np.ascontiguousarray(hidden_states, dtype=np.float32)
    if attention_mask is not None and np.any(attention_mask):
        return _kernel_np(hidden_states, attention_mask, Wq, bq, Wk, bk, Wv, bv)

    from concourse.bass_utils import run_bass_kernel_spmd

    nc = _get_nc()
    ones = np.ones(NTCH * NHEADS_CORE * P, bf)
    hs_bf = hidden_states.astype(bf)
    wq_bf = np.asarray(Wq, np.float32).astype(bf)
    wk_bf = np.asarray(Wk, np.float32).astype(bf)
    wv_bf = np.asarray(Wv, np.float32).astype(bf)
    in_maps = []
    for core in range(N_CORES):
        b = core // 4
        hg = core % 4
        c0 = hg * HG
        in_maps.append({
            "hs": np.ascontiguousarray(hs_bf[:, b, :]),
            "wq": np.ascontiguousarray(wq_bf[:, c0:c0 + HG]),
            "wk": np.ascontiguousarray(wk_bf[:, c0:c0 + HG]),
            "wv": np.ascontiguousarray(wv_bf[:, c0:c0 + HG]),
            "bq": np.ascontiguousarray(bq[c0:c0 + HG], dtype=np.float32),
            "bk": np.ascontiguousarray(bk[c0:c0 + HG], dtype=np.float32),
            "bv": np.ascontiguousarray(bv[c0:c0 + HG], dtype=np.float32),
            "ones": ones,
        })
    res = run_bass_kernel_spmd(
        nc, in_maps, core_ids=list(range(N_CORES)), trace=_trace, tmpdir=_tmpdir
    )
    out = np.empty((S, B, H), np.float32)
    for core in range(N_CORES):
        b = core // 4
        hg = core % 4
        out[:, b, hg * HG:(hg + 1) * HG] = res.results[core]["out"]
    if _trace:
        _CACHE["last_results"] = res
    return out


# revision 18
# speedup vs baseline: 1.1493x; 1.1493x over previous
"""BERT self-attention kernel for Trainium2, 8-core SPMD.

Problem: hidden_states [S=2048, B=2, H=1024], 16 heads x 64, fp32.
Sharding: core i handles batch b = i//4 and head-group hg = i%4
(4 heads = 256 contiguous columns of Wq/Wk/Wv). Each core:

  hsT   = hs.T                          (PE transposes, f32 -> bf16)
  qT/kT = W.T @ hsT (+bias on DVE)      [d, s] bf16
  v     = hsT.T @ Wv (+bias, K=1 mm)    [t, d] bf16, + ones col
  scT   = kT_h.T @ qT_h                 [t, s] bf16 K=64; both heads of a
                                        pair issued back-to-back at PE
                                        row offsets 0/64 (tile_position)
  expT  = exp(scT / 8)                  (ScalarE, scale fused, bf16 out)
  ctxT_aug = v_aug.T @ expT             [65, s] f32 psum; row 64 = sumexp
  out   = transpose(ctxT_aug)[:, 0:64] * (1 / col 64)

Softmax normalization is deferred past the PV matmul (softmax is
shift-invariant and scores are O(1) here, so no max-subtraction).
The kernel is emitted as a single software pipeline: hs DMA ->
transpose -> K0/Q0 projections -> pair-0 attention (with V and the
pair-1 projections woven into its slack) -> pair-1 attention. The
ScalarE exp stream (~135us) is the critical resource; everything else
hides under it.
"""

import numpy as np

S = 2048
B = 2
H = 1024
NH = 16
HD = 64
P = 128
HG = 256          # head-group width (4 heads) per core
NHEADS_CORE = 4
SBLK = 512        # query block
NB = S // SBLK    # 4
NTCH = S // P     # 16 key chunks
KO = H // P       # 8 contraction chunks for projections
N_CORES = 8

_CACHE = {}


def _build_nc():
    import concourse.mybir as mybir
    import concourse.tile as tile
    from concourse import bacc
    from concourse.masks import make_identity

    f32 = mybir.dt.float32
    bf16 = mybir.dt.bfloat16
    Exp = mybir.ActivationFunctionType.Exp

    nc = bacc.Bacc(None, target_bir_lowering=False)

    hs_d = nc.dram_tensor("hs", [S, H], bf16, kind="ExternalInput")
    wq_d = nc.dram_tensor("wq", [H, HG], bf16, kind="ExternalInput")
    wk_d = nc.dram_tensor("wk", [H, HG], bf16, kind="ExternalInput")
    wv_d = nc.dram_tensor("wv", [H, HG], bf16, kind="ExternalInput")
    bq_d = nc.dram_tensor("bq", [HG], f32, kind="ExternalInput")
    bk_d = nc.dram_tensor("bk", [HG], f32, kind="ExternalInput")
    bv_d = nc.dram_tensor("bv", [HG], f32, kind="ExternalInput")
    ones_d = nc.dram_tensor("ones", [NTCH * NHEADS_CORE * P], bf16, kind="ExternalInput")
    out_d = nc.dram_tensor("out", [S, HG], f32, kind="ExternalOutput")

    with tile.TileContext(nc) as tc:
        with (
            tc.tile_pool(name="const", bufs=1) as cst,
            tc.tile_pool(name="qkv", bufs=1) as qkv,
        ):
            ident = cst.tile([P, P], f32)
            make_identity(nc, ident[:])
            # q/k biases as per-partition columns [128, 2] (chunk m on axis 1)
            bcol_q = cst.tile([P, 2], f32)
            nc.sync.dma_start(bcol_q[:], bq_d.rearrange("(m p) -> p m", p=P))
            bcol_k = cst.tile([P, 2], f32)
            nc.sync.dma_start(bcol_k[:], bk_d.rearrange("(m p) -> p m", p=P))
            # v bias row + bf16 ones row for the K=1 bias matmul
            bv_row = cst.tile([1, HG], bf16)
            nc.gpsimd.dma_start(bv_row[:], bv_d[None, :])
            ones_row = cst.tile([1, P], bf16)
            nc.sync.dma_start(ones_row[:], ones_d[None, 0:P])

            # k0/k1 full [d, s]; q split per s-block; v split by t-group
            qkT = {}
            for nm in ("k0", "k1"):
                qkT[nm] = qkv.tile([P, S], bf16, tag=f"T{nm}", name=f"T{nm}")
            qT_s = {}
            for pair in range(2):
                for si in range(NB):
                    qT_s[(pair, si)] = qkv.tile(
                        [P, SBLK], bf16, tag=f"qT{pair}{si}", name=f"qT{pair}{si}"
                    )
            v_g = []
            for g4 in range(NB):
                vt = qkv.tile([P, 4, NHEADS_CORE, HD + 2], bf16,
                              tag=f"v{g4}", name=f"v{g4}")
                nc.sync.dma_start(
                    vt[:, :, :, HD:HD + 1],
                    ones_d.rearrange("(to h p) -> p to h", p=P, to=NTCH)
                    [:, 4 * g4:4 * g4 + 4, :, None],
                )
                v_g.append(vt)

            # Phase-D pools (allocated up front, released at the end)
            ep = tc.alloc_tile_pool(name="expt", bufs=2)
            op = tc.alloc_tile_pool(name="outs", bufs=3)
            scp = tc.alloc_tile_pool(name="sc_ps", bufs=3, space="PSUM")
            cxp = tc.alloc_tile_pool(name="cx_ps", bufs=2, space="PSUM")

            out_v = out_d.rearrange("(nb c p) hh -> p nb c hh", p=P, c=NB)

            with tc.tile_pool(name="hst", bufs=1) as hstp:
                hsT_q = [hstp.tile([P, KO, SBLK], bf16, tag=f"hsT{si}",
                                   name=f"hsT{si}") for si in range(NB)]
                with tc.tile_pool(name="stage", bufs=4) as stp:
                    hs_v = hs_d.rearrange("(so p) h -> so p h", p=P)
                    st_bf = []
                    for so in range(NTCH):
                        st = stp.tile([P, H], bf16, tag="st", name="st")
                        nc.sync.dma_start(st[:], hs_v[so])
                        st_bf.append(st)
                    ident_bf = cst.tile([P, P], bf16)
                    nc.vector.tensor_copy(ident_bf[:], ident[:])

                    def transpose_so(so):
                        tp = cxp.tile([P, KO, P], bf16, tag="cx", name="tp")
                        for ho in range(KO):
                            nc.tensor.transpose(
                                tp[:, ho, :], st_bf[so][:, ho * P:(ho + 1) * P],
                                ident_bf[:],
                            )
                        nc.vector.tensor_copy(
                            hsT_q[so // 4][:, :, (so % 4) * P:(so % 4 + 1) * P],
                            tp[:],
                        )

                    w_sb = {}
                    for name, wd in (("k", wk_d), ("q", wq_d), ("v", wv_d)):
                        w_sb[name] = cst.tile([P, KO, HG], bf16, tag=f"w{name}",
                                              name=f"w{name}")
                        nc.sync.dma_start(
                            w_sb[name][:], wd.rearrange("(ko p) m -> p ko m", p=P)
                        )

                    Ident = mybir.ActivationFunctionType.Identity

                    def qk_proj(w, bcol, m, si, dst, use_act=False):
                        pst = scp.tile([P, 2, SBLK], f32, tag="sc",
                                       name="qk_ps")[:, 0, :]
                        for ko in range(KO):
                            nc.tensor.matmul(
                                pst,
                                w[:, ko, m * P:(m + 1) * P],
                                hsT_q[si][:, ko, :],
                                start=(ko == 0), stop=(ko == KO - 1),
                            )
                        if use_act:
                            nc.scalar.activation(dst, pst, Ident,
                                                 bias=bcol[:, m:m + 1])
                        else:
                            nc.vector.tensor_scalar_add(dst, pst, bcol[:, m:m + 1])

                    def v_proj(to):
                        pst = scp.tile([P, 2, SBLK], f32, tag="sc",
                                       name="v_ps")[:, 0, 0:HG]
                        for ko in range(KO):
                            nc.tensor.matmul(
                                pst,
                                hsT_q[to // 4][:, ko, (to % 4) * P:(to % 4 + 1) * P],
                                w_sb["v"][:, ko, :],
                                start=(ko == 0), stop=False,
                            )
                        nc.tensor.matmul(
                            pst, ones_row[0:1, :], bv_row[:],
                            start=False, stop=True,
                        )
                        nc.vector.tensor_copy(
                            v_g[to // 4][:, to % 4, :, 0:HD],
                            pst.rearrange("p (h d) -> p h d", d=HD),
                        )

                    # ---- attention pipeline ----------------------------
                    def _attention_pair(pair, unit_hook=None):
                        kTt = qkT[f"k{pair}"]
                        for sb_i in range(NB):
                            qTt = qT_s[(pair, sb_i)]
                            expt = ep.tile([P, NTCH, 2, SBLK], bf16, tag="expt",
                                           name="expt")
                            ctxps = [cxp.tile([HD + 1, SBLK], f32, tag="cx",
                                              name=f"ctx{h2}") for h2 in range(2)]

                            def scores_exp(t):
                                sc = scp.tile([P, 2, SBLK], f32, tag="sc",
                                              name="sc")
                                for h2 in range(2):
                                    po = 64 * h2
                                    nc.tensor.matmul(
                                        sc[:, h2, :],
                                        kTt[po:po + HD, t * P:(t + 1) * P],
                                        qTt[po:po + HD, :],
                                        start=True, stop=True,
                                        tile_position=(po, 0),
                                    )
                                nc.scalar.activation(
                                    expt[:, t, :, :], sc[:], Exp, scale=0.125,
                                )

                            def ctx_batch(ts):
                                for h2 in range(2):
                                    head = pair * 2 + h2
                                    for t in ts:
                                        nc.tensor.matmul(
                                            ctxps[h2][:],
                                            v_g[t // 4][:, t % 4, head, 0:HD + 1],
                                            expt[:, t, h2, :],
                                            start=(t == 0), stop=(t == NTCH - 1),
                                            skip_group_check=True,
                                        )

                            for t in range(NTCH):
                                scores_exp(t)
                                if unit_hook is not None:
                                    unit_hook(sb_i, t)
                                if t in (5, 9, 13):
                                    ctx_batch(range(t - 5, t - 1))
                            ctx_batch(range(12, NTCH))

                            for h2 in range(2):
                                head = pair * 2 + h2
                                ctxT = op.tile([HD + 1, SBLK], f32, tag="ctxT",
                                               name="ctxT")
                                nc.vector.tensor_copy(ctxT[:], ctxps[h2][:])
                                ot = cxp.tile([P, NB, HD + 1], f32, tag="cx",
                                              name="ot")
                                for c in range(NB):
                                    nc.tensor.transpose(
                                        ot[:, c, :],
                                        ctxT[:, c * P:(c + 1) * P],
                                        ident[0:HD + 1, 0:HD + 1],
                                    )
                                rec = op.tile([P, NB, 1], f32, tag="rec",
                                              name="rec")
                                nc.vector.reciprocal(rec[:], ot[:, :, HD:HD + 1])
                                osb = op.tile([P, NB, HD], f32, tag="osb",
                                              name="osb")
                                nc.vector.tensor_tensor(
                                    osb[:], ot[:, :, 0:HD],
                                    rec.to_broadcast([P, NB, HD]),
                                    mybir.AluOpType.mult,
                                )
                                nc.sync.dma_start(
                                    out_v[:, sb_i, :, head * HD:(head + 1) * HD],
                                    osb[:],
                                )

                    # ---- emission ---------------------------------------
                    for si in range(NB):
                        for so in range(4 * si, 4 * si + 4):
                            transpose_so(so)
                        qk_proj(w_sb["k"], bcol_k, 0, si,
                                qkT["k0"][:, si * SBLK:(si + 1) * SBLK],
                                use_act=True)
                        qk_proj(w_sb["q"], bcol_q, 0, si, qT_s[(0, si)][:],
                                use_act=True)

                    def _hook_p0(sb_i, t):
                        if sb_i == 0 and 1 <= t <= 8:
                            # two V chains per unit; group g is complete
                            # before ctx needs it two units later
                            v_proj(2 * (t - 1))
                            v_proj(2 * (t - 1) + 1)
                        elif sb_i == 1 and 1 <= t <= 4:
                            si = t - 1
                            qk_proj(w_sb["k"], bcol_k, 1, si,
                                    qkT["k1"][:, si * SBLK:(si + 1) * SBLK])
                        elif sb_i == 2 and 1 <= t <= 4:
                            qk_proj(w_sb["q"], bcol_q, 1, t - 1, qT_s[(1, t - 1)][:])

                    _attention_pair(0, _hook_p0)
                    _attention_pair(1)

            for _pool in (cxp, scp, op, ep):
                _pool.release()
    nc.compile()
    return nc


def _get_nc():
    if "nc" not in _CACHE:
        _CACHE["nc"] = _build_nc()
    return _CACHE["nc"]


def _kernel_np(hidden_states, attention_mask, Wq, bq, Wk, bk, Wv, bv):
    """Numpy fallback for the general (non-zero attention_mask) case."""
    S_, B_, H_ = hidden_states.shape
    hd = H_ // NH

    def split(x):
        return x.reshape(S_, B_ * NH, hd).transpose(1, 0, 2)

    q = split(hidden_states @ Wq + bq)
    k = split(hidden_states @ Wk + bk)
    v = split(hidden_states @ Wv + bv)
    scores = np.einsum("nsd,ntd->nst", q, k).reshape(B_, NH, S_, S_)
    scores = scores / np.sqrt(np.float32(hd)) + attention_mask
    scores = scores - scores.max(axis=-1, keepdims=True)
    e = np.exp(scores)
    probs = (e / e.sum(axis=-1, keepdims=True)).reshape(B_ * NH, S_, S_)
    ctx = np.einsum("nst,ntd->nsd", probs.astype(np.float32), v)
    return ctx.transpose(1, 0, 2).reshape(S_, B_, H_).astype(np.float32)


def kernel(hidden_states, attention_mask, Wq, bq, Wk, bk, Wv, bv, _trace=False, _tmpdir=None):
    import ml_dtypes
    bf = ml_dtypes.bfloat16
    hidden_states = np.ascontiguousarray(hidden_states, dtype=np.float32)
    if attention_mask is not None and np.any(attention_mask):
        return _kernel_np(hidden_states, attention_mask, Wq, bq, Wk, bk, Wv, bv)

    from concourse.bass_utils import run_bass_kernel_spmd

    nc = _get_nc()
    ones = np.ones(NTCH * NHEADS_CORE * P, bf)
    hs_bf = hidden_states.astype(bf)
    wq_bf = np.asarray(Wq, np.float32).astype(bf)
    wk_bf = np.asarray(Wk, np.float32).astype(bf)
    wv_bf = np.asarray(Wv, np.float32).astype(bf)
    in_maps = []
    for core in range(N_CORES):
        b = core // 4
        hg = core % 4
        c0 = hg * HG
        in_maps.append({
            "hs": np.ascontiguousarray(hs_bf[:, b, :]),
            "wq": np.ascontiguousarray(wq_bf[:, c0:c0 + HG]),
            "wk": np.ascontiguousarray(wk_bf[:, c0:c0 + HG]),
            "wv": np.ascontiguousarray(wv_bf[:, c0:c0 + HG]),
            "bq": np.ascontiguousarray(bq[c0:c0 + HG], dtype=np.float32),
            "bk": np.ascontiguousarray(bk[c0:c0 + HG], dtype=np.float32),
            "bv": np.ascontiguousarray(bv[c0:c0 + HG], dtype=np.float32),
            "ones": ones,
        })
    res = run_bass_kernel_spmd(
        nc, in_maps, core_ids=list(range(N_CORES)), trace=_trace, tmpdir=_tmpdir
    )
    out = np.empty((S, B, H), np.float32)
    for core in range(N_CORES):
        b = core // 4
        hg = core % 4
        out[:, b, hg * HG:(hg + 1) * HG] = res.results[core]["out"]
    if _trace:
        _CACHE["last_results"] = res
    return out
